# revision 8
# baseline (speedup 1.0000x reference)
"""Trainium2 Bass kernel for nn_DWAttentionV2 (window conv-attention).

Strategy: data-parallel over batch (16 batches -> 8 cores x 2). Each core runs
an identical single-core NEFF; inputs (x slices + replicated weights, all
host-staged into matmul-friendly layouts) differ per core.

Per batch on device:
  conv3x3(192->768)+relu, 1x1(768->768)+relu, 1x1(768->576)+sigmoid   (PE+DVE+ACT)
  t-layout gather (affine map n = 3*col + e - 1024*c)                  (DVE)
  elementwise multiply with permuted x                                 (DVE)
  DRAM-roundtrip reinterpret -> U layouts, PE transposes for Q/K       (DMA+PE)
  attention: S^T = K Q^T (K=16, row-packed), exp on ACT (scale=0.25,
  no max-subtraction -- |S*scale| <= ~9), PV with ones-column for the
  softmax denominators (M=32, col-packed), recip + G-matmul broadcast,
  normalization fused into psum->sbuf multiply                          (PE+ACT+DVE)
  output projection with zero-padded messy-layout w_out                 (PE)
"""

import os
import sys
from contextlib import ExitStack

import numpy as np
import ml_dtypes

sys.path.insert(0, "/opt/trn_rl_repo")

import concourse.bass as bass
import concourse.bacc as bacc
import concourse.mybir as mybir
import concourse.tile as tile
from concourse.bass_utils import run_bass_kernel_spmd

BF16 = mybir.dt.bfloat16
F32 = mybir.dt.float32
AF = mybir.ActivationFunctionType
ALU = mybir.AluOpType

P = 32
N = 1024          # positions per window
C = 192
HEADS = 12
HD = 16
CH = 768          # hidden conv channels
C3 = 576          # 3*C
B_LOC = 2         # batches per core
N_CORES = 8
SCALE = HD ** -0.5


def _bf(a):
    return np.ascontiguousarray(np.asarray(a, dtype=np.float32).astype(ml_dtypes.bfloat16))


def _f32(a):
    return np.ascontiguousarray(np.asarray(a, dtype=np.float32))


def _host_weights(w1, b1, w2, b2, w3, b3, w_out):
    """Host-side weight staging into device layouts (layout prep only)."""
    w1 = _f32(w1); w2 = _f32(w2); w3 = _f32(w3); w_out = _f32(w_out)
    # conv1 lhsT: per offset o=3*ky+kx, [ic, oc]; split ic into 128 + 64
    w1t = w1.transpose(2, 3, 1, 0).reshape(9, 192, 768)      # [o, ic, oc]
    w1a = w1t[:, :128].transpose(1, 0, 2).reshape(128, 9 * 768)
    w1b = w1t[:, 128:].transpose(1, 0, 2).reshape(64, 9 * 768)
    # conv2 lhsT: [k, p, oc] -> [128, 6*768]
    w2t = w2[:, :, 0, 0].T.reshape(6, 128, 768).transpose(1, 0, 2).reshape(128, 6 * 768)
    # conv3 lhsT: [k, p, m(576)] -> [128, 6*576]
    w3t = w3[:, :, 0, 0].T.reshape(6, 128, 576).transpose(1, 0, 2).reshape(128, 6 * 576)
    b1s = _f32(b1).reshape(6, 128).T.copy()
    b2s = _f32(b2).reshape(6, 128).T.copy()
    b3s = _f32(b3).reshape(6, 96).T.copy()
    ident = np.eye(128, dtype=np.float32)
    # G': row 32j+16 broadcast to rows 32j..32j+16 (within each 32-group)
    gsel = np.zeros((128, 128), np.float32)
    for j in range(4):
        gsel[32 * j, 32 * j:32 * j + 18] = 1.0
    # messy-layout w_out rhs: [128, 3*192]; rows 32j+k of group g = head 4g+j
    woutm = np.zeros((128, 3 * 192), np.float32)
    for g in range(3):
        for j in range(4):
            h = 4 * g + j
            for k in range(16):
                woutm[32 * j + 1 + k, g * 192:(g + 1) * 192] = w_out[:, 16 * h + k]
    return {
        "w1a": _bf(w1a), "w1b": _bf(w1b), "w2t": _bf(w2t), "w3t": _bf(w3t),
        "b1s": b1s, "b2s": b2s, "b3s": b3s,
        "ident": _bf(ident), "gsel": gsel, "woutm": _bf(woutm),
    }


def _host_x(x_core):
    """Stage a core's x slice [B_LOC, 1024, 192] into conv / t-layout forms."""
    xpad = np.zeros((B_LOC, 192, 34, 34), np.float32)
    ty = np.zeros((B_LOC, 192, 1024), np.float32)
    for b in range(B_LOC):
        xi = x_core[b].T.astype(np.float32)              # [192, 1024]
        xpad[b, :, 1:33, 1:33] = xi.reshape(192, 32, 32)
        ty[b] = xi.flatten().reshape(1024, 192).T        # t-layout of y
    return {"xpad": _bf(xpad.reshape(B_LOC, 192, 34 * 34)), "ty": _bf(ty)}


# --------------------------------------------------------------------------
# device kernel build
# --------------------------------------------------------------------------

def build_nc():
    nc = bacc.Bacc("TRN2", target_bir_lowering=False, debug=False,
                   num_devices=N_CORES)

    din = {}
    def dram_in(name, shape, dt):
        din[name] = nc.dram_tensor(name, shape, dt, kind="ExternalInput").ap()

    dram_in("xpad", [B_LOC, 192, 1156], BF16)
    dram_in("ty", [B_LOC, 192, 1024], BF16)
    dram_in("w1a", [128, 9 * 768], BF16)
    dram_in("w1b", [64, 9 * 768], BF16)
    dram_in("w2t", [128, 6 * 768], BF16)
    dram_in("w3t", [128, 6 * 576], BF16)
    dram_in("b1s", [128, 6], F32)
    dram_in("b2s", [128, 6], F32)
    dram_in("b3s", [96, 6], F32)
    dram_in("ident", [128, 128], BF16)
    dram_in("gsel", [128, 128], F32)
    dram_in("woutm", [128, 3 * 192], BF16)
    out_d = nc.dram_tensor("out", [B_LOC, 1024, 192], F32, kind="ExternalOutput").ap()

    with tile.TileContext(nc) as tc:
        _build_body(tc, din, out_d)
    nc.compile()
    return nc


def _build_body(tc, din, out_d):
    nc = tc.nc
    sync = nc.sync

    ctx = ExitStack()
    persist = ctx.enter_context(tc.tile_pool(name="persist", bufs=1))
    psp = ctx.enter_context(tc.tile_pool(name="psum", bufs=6, space="PSUM"))
    dramp = ctx.enter_context(tc.tile_pool(name="drams", bufs=2, space="DRAM"))

    def ptile(tag, bufs=6, dt=F32):
        return psp.tile([128, 512], dt, tag=tag, bufs=bufs, name=tag)

    # ---- persistent weight loads ----
    sb = {}
    for name, shape, dt in [
        ("w1a", [128, 9 * 768], BF16), ("w1b", [64, 9 * 768], BF16),
        ("w2t", [128, 6 * 768], BF16), ("w3t", [128, 6 * 576], BF16),
        ("b1s", [128, 6], F32), ("b2s", [128, 6], F32), ("b3s", [96, 6], F32),
        ("ident", [128, 128], BF16), ("gsel", [128, 128], F32),
        ("woutm", [128, 3 * 192], BF16),
    ]:
        t = persist.tile(shape, dt, tag=name, name=name)
        sync.dma_start(out=t[:], in_=din[name])
        sb[name] = t

    # persistent U-layout tiles (32-stride heads), zeroed once
    uq = persist.tile([128, 8 * 384], BF16, tag="uq", name="uq")
    uk = persist.tile([128, 8 * 384], BF16, tag="uk", name="uk")
    uv = persist.tile([128, 8 * 384], BF16, tag="uv", name="uv")
    for t in (uq, uk, uv):
        nc.gpsimd.memset(t[:], 0.0)
    # ones column for the softmax denominators: col 32h+16 of each mt block
    uv4 = uv[:].rearrange("p (m h x) -> p m h x", m=8, h=12)
    nc.gpsimd.memset(uv4[:, :, :, 0:1], 1.0)

    # messy recip tiles (one per head-quad g), rows 32j+16 hold 1/denom
    recipm = []
    for g in range(3):
        t = persist.tile([128, 1024], F32, tag=f"recipm{g}", name=f"recipm{g}")
        nc.gpsimd.memset(t[:], 0.0)
        recipm.append(t)

    for b in range(B_LOC):
        convs = ExitStack()
        cp = convs.enter_context(tc.tile_pool(name=f"conv{b}", bufs=1))

        # ---- conv phase ----
        xp0 = cp.tile([128, 1156], BF16, tag="xp0", name="xp0")
        xp1 = cp.tile([64, 1156], BF16, tag="xp1", name="xp1")
        sync.dma_start(out=xp0[:], in_=din["xpad"][b, 0:128, :])
        sync.dma_start(out=xp1[:], in_=din["xpad"][b, 128:192, :])
        xv0 = xp0[:].rearrange("p (r c) -> p r c", c=34)
        xv1 = xp1[:].rearrange("p (r c) -> p r c", c=34)

        a1 = [cp.tile([128, 1024], BF16, tag=f"a1_{t}", name=f"a1_{t}") for t in range(6)]
        a2 = [cp.tile([128, 1024], BF16, tag=f"a2_{t}", name=f"a2_{t}") for t in range(6)]
        a3 = [cp.tile([96, 1024], BF16, tag=f"a3_{t}", name=f"a3_{t}") for t in range(6)]

        # conv1: 3x3, K = 192(=128+64) per offset, accumulate 18 matmuls
        for mt in range(6):
            for h2 in range(2):
                ps = ptile("ps")
                first = True
                for ky in range(3):
                    for kx in range(3):
                        o = 3 * ky + kx
                        rhs0 = xv0[:, ky + 16 * h2: ky + 16 * h2 + 16, kx:kx + 32]
                        rhs1 = xv1[:, ky + 16 * h2: ky + 16 * h2 + 16, kx:kx + 32]
                        lhs0 = sb["w1a"][:, o * 768 + 128 * mt: o * 768 + 128 * mt + 128]
                        lhs1 = sb["w1b"][:, o * 768 + 128 * mt: o * 768 + 128 * mt + 128]
                        nc.tensor.matmul(ps[:], lhs0, rhs0, start=first, stop=False)
                        first = False
                        last = (ky == 2 and kx == 2)
                        nc.tensor.matmul(ps[:], lhs1, rhs1, start=False, stop=last)
                nc.vector.tensor_scalar(
                    out=a1[mt][:, 512 * h2: 512 * h2 + 512], in0=ps[:],
                    scalar1=sb["b1s"][:, mt:mt + 1], scalar2=0.0,
                    op0=ALU.add, op1=ALU.max)

        # conv2: 1x1 768->768
        for mt in range(6):
            for h2 in range(2):
                ps = ptile("ps")
                for k in range(6):
                    nc.tensor.matmul(
                        ps[:], sb["w2t"][:, k * 768 + 128 * mt: k * 768 + 128 * mt + 128],
                        a1[k][:, 512 * h2: 512 * h2 + 512],
                        start=(k == 0), stop=(k == 5))
                nc.vector.tensor_scalar(
                    out=a2[mt][:, 512 * h2: 512 * h2 + 512], in0=ps[:],
                    scalar1=sb["b2s"][:, mt:mt + 1], scalar2=0.0,
                    op0=ALU.add, op1=ALU.max)

        # conv3: 1x1 768->576 (96-row M tiles) + sigmoid
        for mt in range(6):
            for h2 in range(2):
                ps = ptile("ps")
                for k in range(6):
                    nc.tensor.matmul(
                        ps[0:96, :], sb["w3t"][:, k * 576 + 96 * mt: k * 576 + 96 * mt + 96],
                        a2[k][:, 512 * h2: 512 * h2 + 512],
                        start=(k == 0), stop=(k == 5))
                nc.scalar.activation(
                    a3[mt][:, 512 * h2: 512 * h2 + 512], ps[0:96, :], AF.Sigmoid,
                    bias=sb["b3s"][:, mt:mt + 1])

        # ---- t-layout build + multiply + roundtrip out ----
        tps = ExitStack()
        tp = tps.enter_context(tc.tile_pool(name=f"tz{b}", bufs=1))
        tyt = [tp.tile([96, 1024], BF16, tag=f"ty{i}", name=f"ty{i}") for i in range(2)]
        sync.dma_start(out=tyt[0][:], in_=din["ty"][b, 0:96, :])
        sync.dma_start(out=tyt[1][:], in_=din["ty"][b, 96:192, :])

        zbuf = [dramp.tile([192 * 1024], BF16, tag=f"zbuf{c}", name=f"zbuf{c}") for c in range(3)]
        for c in range(3):
            ta = [tp.tile([96, 1026], BF16, tag=f"ta{c}_{i}", name=f"ta{c}_{i}") for i in range(2)]
            # gather: T_a_c[ch', 3*col + e - 1024c] = a3[192e + ch', col]
            for e in range(3):
                nlo = 1024 * c - e
                col0 = -(-nlo // 3) if nlo > 0 else 0
                col1 = (1023 + 1024 * c - e) // 3
                col1 = min(col1, 1023)
                cnt = col1 - col0 + 1
                n0 = 3 * col0 + e - 1024 * c
                r = n0 % 3
                a0 = (n0 - r) // 3
                for i in range(2):
                    dst = ta[i][:].rearrange("p (a r) -> p a r", r=3)
                    nc.vector.tensor_copy(
                        dst[:, a0:a0 + cnt, r],
                        a3[2 * e + i][:, col0:col0 + cnt])
            tz = [tp.tile([96, 1024], BF16, tag=f"tz{c}_{i}", name=f"tzt{c}_{i}") for i in range(2)]
            for i in range(2):
                nc.vector.tensor_mul(tz[i][:], ta[i][:, 0:1024], tyt[i][:])
            zv = zbuf[c][:].rearrange("(p n) -> p n", p=192)
            sync.dma_start(out=zv[0:96, :], in_=tz[0][:])
            sync.dma_start(out=zv[96:192, :], in_=tz[1][:])
        tps.close()
        convs.close()

        # ---- roundtrip in: U layouts (32-stride heads) ----
        for c, udst in ((0, uq), (1, uk), (2, uv)):
            lo = 1 if c == 2 else 0
            zu = zbuf[c][:].rearrange("(n c) -> n c", n=1024)
            uview = udst[:].rearrange("p (m h x) -> p m h x", m=8, h=12)
            for mt in range(8):
                src = zu[128 * mt:128 * mt + 128, :].rearrange(
                    "p (h x) -> p h x", h=12)
                sync.dma_start(out=uview[:, mt, :, lo:lo + 16], in_=src)

        attns = ExitStack()
        ap_ = attns.enter_context(tc.tile_pool(name=f"attn{b}", bufs=1))
        expp = attns.enter_context(tc.tile_pool(name=f"exp{b}", bufs=10))

        # ---- PE transposes: U_q/U_k -> U^T (c on partitions, 32-stride) ----
        uqT = [ap_.tile([128, 1024], BF16, tag=f"uqT{t}", name=f"uqT{t}") for t in range(3)]
        ukT = [ap_.tile([128, 1024], BF16, tag=f"ukT{t}", name=f"ukT{t}") for t in range(3)]
        for usrc, udstT in ((uq, uqT), (uk, ukT)):
            for t in range(3):
                for mq in range(2):
                    ps = ptile("ps", dt=BF16)
                    for j in range(4):
                        mt = 4 * mq + j
                        nc.tensor.transpose(
                            ps[:, 128 * j:128 * j + 128],
                            usrc[:, mt * 384 + 128 * t: mt * 384 + 128 * t + 128],
                            sb["ident"][:])
                    nc.vector.tensor_copy(
                        udstT[t][:, 512 * mq:512 * mq + 512], ps[:])

        # ---- attention ----
        otm = [ap_.tile([128, 1024], BF16, tag=f"otm{g}", name=f"otm{g}") for g in range(3)]
        for half in range(2):
            for t in range(3):          # head quad (also PV psum group)
                pvps = ptile("pv", bufs=2)
                for mt in range(8):
                    qk = []
                    for j in range(4):
                        ps = ptile("ps")
                        nc.tensor.matmul(
                            ps[:],
                            ukT[t][32 * j:32 * j + 16, 128 * mt:128 * mt + 128],
                            uqT[t][32 * j:32 * j + 16, 512 * half:512 * half + 512],
                            start=True, stop=True,
                            tile_position=(32 * j, 0))
                        qk.append(ps)
                    for j in range(4):
                        ex = expp.tile([128, 512], BF16, tag="expS", name="expS")
                        nc.scalar.activation(ex[:], qk[j][:], AF.Exp, scale=SCALE)
                        qk[j] = ex
                    for j in range(4):
                        h = 4 * t + j
                        nc.tensor.matmul(
                            pvps[32 * j:32 * j + 32, :],
                            uv[:, mt * 384 + 32 * h: mt * 384 + 32 * h + 32],
                            qk[j][:],
                            start=(mt == 0), stop=(mt == 7),
                            tile_position=(0, 32 * j), skip_group_check=True)
                # denominators -> recip (rows 32j+16), then R broadcast matmul
                for j in range(4):
                    nc.vector.reciprocal(
                        out=recipm[t][32 * j:32 * j + 1,
                                      512 * half:512 * half + 512],
                        in_=pvps[32 * j:32 * j + 1, :])
                rps = ptile("ps")
                nc.tensor.matmul(rps[:], sb["gsel"][:],
                                 recipm[t][:, 512 * half:512 * half + 512],
                                 start=True, stop=True)
                rsb = expp.tile([128, 512], F32, tag="rsb", bufs=2, name="rsb")
                nc.vector.tensor_copy(rsb[:], rps[:])
                nc.vector.tensor_mul(
                    otm[t][:, 512 * half:512 * half + 512], pvps[:], rsb[:])

        # ---- output projection ----
        for n2c in range(8):
            ps = ptile("ps")
            for g in range(3):
                nc.tensor.matmul(
                    ps[:, 0:192], otm[g][:, 128 * n2c:128 * n2c + 128],
                    sb["woutm"][:, g * 192:(g + 1) * 192],
                    start=(g == 0), stop=(g == 2))
            osb = ap_.tile([128, 192], F32, tag="osb", name="osb")
            nc.vector.tensor_copy(osb[:], ps[:, 0:192])
            sync.dma_start(out=out_d[b, 128 * n2c:128 * n2c + 128, :], in_=osb[:])

        attns.close()

    ctx.close()


# --------------------------------------------------------------------------
# host entry
# --------------------------------------------------------------------------

_NC_CACHE = None


def kernel(x, w1, b1, w2, b2, w3, b3, w_out):
    global _NC_CACHE
    x = np.asarray(x)
    B = x.shape[0]
    assert B == B_LOC * N_CORES, f"expected B={B_LOC * N_CORES}, got {B}"

    wmap = _host_weights(w1, b1, w2, b2, w3, b3, w_out)
    if _NC_CACHE is None:
        _NC_CACHE = build_nc()
    nc = _NC_CACHE

    in_maps = []
    for core in range(N_CORES):
        m = dict(wmap)
        m.update(_host_x(x[B_LOC * core:B_LOC * (core + 1)]))
        in_maps.append(m)

    trace = os.environ.get("KERNEL_TRACE", "0") == "1"
    res = run_bass_kernel_spmd(nc, in_maps, core_ids=list(range(N_CORES)),
                               trace=trace)
    kernel.last_results = res
    out = np.concatenate([r["out"] for r in res.results], axis=0)
    return out.astype(np.float32)
